# revision 8
# baseline (speedup 1.0000x reference)
"""Trainium2 Bass kernel for nn_ADNN_57501022159565 (GNN message passing).

Reference computation (B=4096, D=256, N=128, F=128, O=256):
    conn = softmax(attention, axis=1)                      # [N, N]
    W_s  = node_weight[:, :F, :]                           # [N, F, F]
    W_x  = node_weight[:, F:, :]                           # [N, D, F]
    X    = einsum('bd,ndf->nbf', x, W_x) + bias[:,None,:]  # [N, B, F]
    S_1  = relu(X)                                         # (states0 = 0)
    S_k  = relu(einsum('ij,jbf->ibf', conn, S_{k-1}) @ W_s + X)
    out  = mean_n(S_T) @ out_w + out_b                     # [B, O]

Strategy: pure data-parallel over batch, B=4096 split across 8 NeuronCores
(512 rows each), weights replicated, no collectives. Compute in bf16 with
f32 PSUM accumulation. Per-core batch processed in 4 chunks of 128.

Layouts (per chunk, SBUF):
  S_nm  (j | f, bs)  node-major rows   -> agg lhsT slices [j, f] at fixed b
  agg   (f | i, bs)  feature-major     <- strided PSUM evictions of agg matmuls
  X_sb  (f | n, bs)  feature-major     (xW + bias, pre-relu; reused each iter)
The per-iteration feature-major -> node-major rearrange is done with one
SBUF->SBUF DMA per node (tile -> partition row), fused into the mm2 epilogue.
"""
import sys

if "/opt/trn_rl_repo" not in sys.path:
    sys.path.insert(0, "/opt/trn_rl_repo")

import numpy as np

import concourse.bass as bass
import concourse.tile as tile
from concourse import bacc, mybir
from concourse.bass_utils import run_bass_kernel_spmd
from concourse.masks import make_identity

F32 = mybir.dt.float32
BF16 = mybir.dt.bfloat16
AF = mybir.ActivationFunctionType

B, D, N, F, O = 4096, 256, 128, 128, 256
NCORES = 8
BLOC = B // NCORES          # 512 batch rows per core
BS = 128                    # batch chunk
NCHUNK = BLOC // BS         # 4


def build_nc(num_iterations: int, reps: int = 1):
    T = int(num_iterations)
    nc = bacc.Bacc(None, target_bir_lowering=False)
    x_ext = nc.declare_dram_parameter("x", [BLOC, D], F32, isOutput=False)
    nw_ext = nc.declare_dram_parameter("node_weight", [N, F + D, F], F32, isOutput=False)
    nb_ext = nc.declare_dram_parameter("node_bias", [N, F], F32, isOutput=False)
    att_ext = nc.declare_dram_parameter("attention", [N, N], F32, isOutput=False)
    ow_ext = nc.declare_dram_parameter("out_w", [F, O], F32, isOutput=False)
    ob_ext = nc.declare_dram_parameter("out_b", [1, O], F32, isOutput=False)
    out_ext = nc.declare_dram_parameter("out", [BLOC, O], F32, isOutput=True)

    nw_r = nw_ext[:].rearrange("n d g -> d n g")  # (D+F, N, F) strided view

    with tile.TileContext(nc) as tc:
        with (
            tc.tile_pool(name="const", bufs=1) as cp,
            tc.tile_pool(name="wp", bufs=1) as wp,
            tc.tile_pool(name="big", bufs=1) as bp,
            tc.tile_pool(name="tmp", bufs=1) as tp,
            tc.tile_pool(name="stage", bufs=4) as sp,
            tc.tile_pool(name="wstage", bufs=2) as wsp,
            tc.tile_pool(name="ps_a", bufs=3, space="PSUM") as ps_a,
            tc.tile_pool(name="ps_m", bufs=3, space="PSUM") as ps_m,
            tc.tile_pool(name="ps_o", bufs=2, space="PSUM") as ps_o,
        ):
            for _rep in range(reps):
                _body(nc, tc, T, cp, wp, bp, tp, sp, wsp, ps_a, ps_m, ps_o,
                      x_ext, nw_r, nb_ext, att_ext, ow_ext, ob_ext, out_ext)
    nc.compile()
    return nc


def _body(nc, tc, T, cp, wp, bp, tp, sp, wsp, ps_a, ps_m, ps_o,
          x_ext, nw_r, nb_ext, att_ext, ow_ext, ob_ext, out_ext):
    # ---------------- setup: constants ----------------
    ident = cp.tile([128, 128], BF16, tag="ident")
    make_identity(nc, ident[:])
    ones = cp.tile([1, 128], BF16, tag="ones")
    nc.gpsimd.memset(ones[:], 1.0)

    # out_w scaled by 1/N (folds the node-mean), out_b
    ow = cp.tile([F, O], BF16, tag="ow")
    nc.gpsimd.dma_start(ow[:], ow_ext[:])  # cast f32->bf16
    nc.vector.tensor_scalar_mul(ow[:], ow[:], 1.0 / N)
    ob = cp.tile([1, O], BF16, tag="ob")
    nc.gpsimd.dma_start(ob[:], ob_ext[:])  # cast f32->bf16

    if T == 0:
        po = ps_o.tile([BS, O], F32, tag="po")
        nc.tensor.matmul(po[:], ones[:, 0:BS], ob[:], start=True, stop=True)
        ot = sp.tile([BS, O], F32, tag="ot", bufs=1)
        nc.vector.tensor_copy(ot[:], po[:])
        for c in range(NCHUNK):
            nc.sync.dma_start(out_ext[c * BS:(c + 1) * BS, :], ot[:])
        return

    # softmax(attention) -> conn (bf16), then connT via PE transpose
    att = tp.tile([N, N], F32, tag="att")
    nc.sync.dma_start(att[:], att_ext[:])
    mx = tp.tile([N, 1], F32, tag="mx")
    nc.vector.tensor_reduce(mx[:], att[:], axis=mybir.AxisListType.X,
                            op=mybir.AluOpType.max, negate=True)
    ex = tp.tile([N, N], F32, tag="ex")
    nc.scalar.activation(ex[:], att[:], AF.Exp, bias=mx[:, 0:1])
    sm = tp.tile([N, 1], F32, tag="sm")
    nc.vector.tensor_reduce(sm[:], ex[:], axis=mybir.AxisListType.X,
                            op=mybir.AluOpType.add)
    rc = tp.tile([N, 1], F32, tag="rc")
    nc.vector.reciprocal(rc[:], sm[:])
    conn = tp.tile([N, N], BF16, tag="conn")
    nc.vector.tensor_scalar_mul(conn[:], ex[:], rc[:, 0:1])
    pt = ps_o.tile([N, N], BF16, tag="po")
    nc.tensor.transpose(pt[:], conn[:], ident[:])
    connT = cp.tile([N, N], BF16, tag="connT")
    nc.vector.tensor_copy(connT[:], pt[:])

    # node_bias -> bias_fm (f | n) f32
    nb_bf = tp.tile([N, F], BF16, tag="nb_bf")
    nc.gpsimd.dma_start(nb_bf[:], nb_ext[:])
    pb = ps_o.tile([F, N], BF16, tag="po")
    nc.tensor.transpose(pb[:], nb_bf[:], ident[:])
    bias_fm = cp.tile([F, N], F32, tag="bias_fm")
    nc.vector.tensor_copy(bias_fm[:], pb[:])

    # x -> xT (d | b) bf16, two d-halves
    xT = [cp.tile([128, BLOC], BF16, tag=f"xT{dc}", name=f"xT{dc}") for dc in range(2)]
    for c in range(NCHUNK):
        xs = sp.tile([BS, D], BF16, tag="mtmp", bufs=2, name="xs")
        nc.gpsimd.dma_start(xs[:], x_ext[c * BS:(c + 1) * BS, :])
        for dc in range(2):
            px = ps_o.tile([128, BS], BF16, tag="po")
            nc.tensor.transpose(px[:], xs[:, dc * 128:(dc + 1) * 128], ident[:])
            nc.vector.tensor_copy(xT[dc][:, c * BS:(c + 1) * BS], px[:])

    # weights: staged f32 loads (HWDGE strided) + DVE cast to bf16
    W_s = wp.tile([F, N, F], BF16, tag="W_s")      # (f | n, g)
    W_x = [wp.tile([128, N, F], BF16, tag=f"W_x{dc}", name=f"W_x{dc}") for dc in range(2)]
    NG = 4
    for base in range(0, N, NG):
        for dc in range(2):
            wst = wsp.tile([128, NG, F], F32, tag="wst")
            nc.sync.dma_start(wst[:], nw_r[F + dc * 128:F + (dc + 1) * 128,
                                           base:base + NG, :])
            nc.vector.tensor_copy(W_x[dc][:, base:base + NG, :], wst[:])
        wst = wsp.tile([128, NG, F], F32, tag="wst")
        nc.sync.dma_start(wst[:], nw_r[0:F, base:base + NG, :])
        nc.vector.tensor_copy(W_s[:, base:base + NG, :], wst[:])

    # ---------------- main pipeline ----------------
    for c in range(NCHUNK):
        cs = slice(c * BS, (c + 1) * BS)
        X_sb = bp.tile([F, N, BS], BF16, tag="X")       # (f | n, bs)
        S_nm = bp.tile([N, F, BS], BF16, tag="S")       # (j | f, bs)
        agg = bp.tile([F, N, BS], BF16, tag="agg")      # (f | i, bs)
        macc = [bp.tile([F, BS], F32, tag=f"macc{a}", name=f"macc{a}") for a in range(2)]

        # phase 1: X = xW + bias ; S_1 = relu(X)
        for n in range(N):
            pm = ps_m.tile([F, BS], F32, tag="pm")
            nc.tensor.matmul(pm[:], W_x[0][:, n, :], xT[0][:, cs],
                             start=True, stop=False)
            nc.tensor.matmul(pm[:], W_x[1][:, n, :], xT[1][:, cs],
                             start=False, stop=True)
            nc.scalar.activation(X_sb[:, n, :], pm[:], AF.Identity,
                                 bias=bias_fm[:, n:n + 1])
            if T >= 2:
                st = sp.tile([F, BS], BF16, tag="st")
                nc.vector.tensor_scalar_max(st[:], X_sb[:, n, :], 0.0)
                nc.sync.dma_start(S_nm[n:n + 1, :, :], st[:])
            else:
                _mean_step(nc, sp, macc, n, X_sb[:, n, :])

        # iterations 2..T
        for k in range(2, T + 1):
            last = (k == T)
            for b in range(BS):
                pa = ps_a.tile([F, N], F32, tag="pa")
                nc.tensor.matmul(pa[:], S_nm[:, :, b], connT[:],
                                 start=True, stop=True)
                nc.any.tensor_copy(agg[:, :, b], pa[:])
            for i in range(N):
                pm = ps_m.tile([F, BS], F32, tag="pm")
                nc.tensor.matmul(pm[:], W_s[:, i, :], agg[:, i, :],
                                 start=True, stop=False)
                nc.tensor.matmul(pm[:], ident[:], X_sb[:, i, :],
                                 start=False, stop=True)
                if not last:
                    st = sp.tile([F, BS], BF16, tag="st")
                    nc.any.tensor_scalar_max(st[:], pm[:], 0.0)
                    nc.sync.dma_start(S_nm[i:i + 1, :, :], st[:])
                else:
                    _mean_step(nc, sp, macc, i, pm[:])

        # combine mean accumulators, final linear, store
        nc.vector.tensor_add(macc[0][:], macc[0][:], macc[1][:])
        mean_bf = sp.tile([F, BS], BF16, tag="mean_bf", bufs=2)
        nc.vector.tensor_copy(mean_bf[:], macc[0][:])
        po = ps_o.tile([BS, O], F32, tag="po")
        nc.tensor.matmul(po[:], mean_bf[:], ow[:], start=True, stop=False)
        nc.tensor.matmul(po[:], ones[:, 0:BS], ob[:], start=False, stop=True)
        ot = sp.tile([BS, O], F32, tag="ot", bufs=1)
        nc.vector.tensor_copy(ot[:], po[:])
        nc.sync.dma_start(out_ext[cs, :], ot[:])


def _mean_step(nc, sp, macc, i, src):
    """macc[i % 4] (+)= relu(src). First touch of each accumulator overwrites."""
    a = macc[i % 2]
    if i < 2:
        nc.vector.tensor_scalar_max(a[:], src, 0.0)
    else:
        tmp = sp.tile([F, BS], F32, tag="mtmp", bufs=2)
        nc.any.tensor_scalar_max(tmp[:], src, 0.0)
        nc.vector.tensor_add(a[:], a[:], tmp[:])


_NC_CACHE = {}


def _get_nc(T: int):
    if T not in _NC_CACHE:
        _NC_CACHE[T] = build_nc(T)
    return _NC_CACHE[T]


def kernel(**inputs) -> np.ndarray:
    x = np.ascontiguousarray(np.asarray(inputs["x"], dtype=np.float32))
    nw = np.ascontiguousarray(np.asarray(inputs["node_weight"], dtype=np.float32))
    nb = np.ascontiguousarray(np.asarray(inputs["node_bias"], dtype=np.float32))
    att = np.ascontiguousarray(np.asarray(inputs["attention"], dtype=np.float32))
    ow = np.ascontiguousarray(np.asarray(inputs["out_w"], dtype=np.float32))
    ob = np.ascontiguousarray(np.asarray(inputs["out_b"], dtype=np.float32)).reshape(1, O)
    T = int(np.asarray(inputs["num_iterations"]))

    nc = _get_nc(T)
    in_maps = []
    for core in range(NCORES):
        shard = x[core * BLOC:(core + 1) * BLOC]
        in_maps.append({
            "x": shard,
            "node_weight": nw,
            "node_bias": nb,
            "attention": att,
            "out_w": ow,
            "out_b": ob,
        })
    res = run_bass_kernel_spmd(nc, in_maps, core_ids=list(range(NCORES)))
    out = np.concatenate([res.results[i]["out"] for i in range(NCORES)], axis=0)
    return out.astype(np.float32)
